# revision 1
# baseline (speedup 1.0000x reference)
"""Correntropy loss on 8 Trainium2 NeuronCores.

Reference math (all f32):
    t = (target - 0.5) * 2 ; o = (output - 0.5) * 2
    cost = mean(1 - exp(-sigma * (o - t)^2)),  sigma = 1/1000

Since o - t == 2*(output - target), this equals
    mean(1 - exp(-c * w)),  w = (output - target)^2,  c = 4*sigma = 0.004

Direct evaluation of sum(1 - exp(-c*w)) on device loses ~3 decimal
digits: the f32 running sums of exp(...) ~= 1 carry a systematic
~2e-7 relative rounding bias that the final N - S cancellation
amplifies ~1500x (c*w <= 0.016, so 1-exp is ~6.6e-4 of each summand).

Instead the device computes exact power sums (moments) of w
    S1 = sum(w), S2 = sum(w^2), S3 = sum(w^3)
and the host evaluates the Taylor series in f64:
    sum(1 - exp(-c*w)) = c*S1 - c^2/2*S2 + c^3/6*S3 - O(c^4*S4)
The dropped S4 term is ~9e-8 relative; every device op involved
(ACT Square LUT, DVE multiply) was verified bit-exact on HW, and the
fused f32 accumulators contribute <~2e-7 (S2/S3 enter scaled by
3e-3 / 9e-6 so their accumulation error is irrelevant).

Sharding (per the data-parallel hint): both tensors row-sharded into
8 x [8192, 1000]; each core's two shards are host-interleaved into one
array [n_tiles, 2, 128, 2000] so one DMA per tile fetches both
operands (fewer cross-engine waits). Per core, 32 tiles of [128x4000]:
    DVE: d  = out_half - tgt_half    (tensor_sub)
    ACT: w  = Square(d),  accum -> S1 column   (bit-exact square)
    ACT: w2 = Square(w),  accum -> S2 column
    DVE: w3 = (w*1)*w2,   accum -> S3 column   (scalar_tensor_tensor)
The last two tiles' compute runs on column slices (DMAs stay full-size)
so the serial chain after the final DMA is short. Partial sums land in
a [128, 108] tile, DMA'd out; host reduces in f64 and applies the
series. The scalar "all-reduce" of the hint happens on the host
(8 tiny [128,108] arrays), which is exact.
"""

import numpy as np

import concourse.bacc as bacc
import concourse.mybir as mybir
import concourse.tile as tile
from concourse.bass_utils import run_bass_kernel_spmd

N_CORES = 8
ROWS = 65536
COLS = 1000
ROWS_PER_CORE = ROWS // N_CORES  # 8192
P = 128  # SBUF partitions

Q = 2  # rows folded into the free dim per partition
FREE = Q * COLS  # 2000 elements of one operand per partition per tile
N_TILES = ROWS_PER_CORE // (P * Q)  # 32

# Tail taper: DMAs stay full-size (per-partition chunks below 8KB stream
# far below line rate), but the COMPUTE of the last two tiles runs on
# column slices so the serial sub->sq->sq->mul chain after the final DMA
# is short (~4us instead of ~10us) - the slices pipeline across engines.
_SLICES = {N_TILES - 2: [(0, 1000), (1000, 1000)],
           N_TILES - 1: [(0, 500), (500, 500), (1000, 500), (1500, 500)]}
# (dram_tile, col_offset, width) compute pieces; widths sum per tile to FREE
PIECES = []
for _t in range(N_TILES):
    for _off, _z in _SLICES.get(_t, [(0, FREE)]):
        PIECES.append((_t, _off, _z))
N_PIECES = len(PIECES)  # 36
ACC_COLS = 3 * N_PIECES  # S1 | S2 | S3 column blocks

F32 = mybir.dt.float32


def _build():
    nc = bacc.Bacc()
    comb_p = nc.declare_dram_parameter(
        "combined", [N_TILES * 2 * P, FREE], F32, isOutput=False
    )
    acc_p = nc.declare_dram_parameter("partial", [P, ACC_COLS], F32, isOutput=True)

    # [n_tiles, 2, P, FREE] -> per-tile [P, 2, FREE] access pattern
    comb_v = comb_p[:].rearrange("(t c p) m -> t p c m", c=2, p=P)

    with tile.TileContext(nc) as tc:
        with (
            tc.tile_pool(name="io", bufs=6) as io_pool,
            tc.tile_pool(name="work", bufs=1) as work_pool,
            tc.tile_pool(name="accp", bufs=1) as acc_pool,
        ):
            acc = acc_pool.tile([P, ACC_COLS], F32)
            ab_tiles = {}
            for i, (t, off, z) in enumerate(PIECES):
                if t not in ab_tiles:
                    ab = io_pool.tile([P, 2 * FREE], F32, tag="ab")
                    nc.sync.dma_start(
                        out=ab[:].rearrange("p (c m) -> p c m", c=2), in_=comb_v[t]
                    )
                    ab_tiles[t] = ab
                ab = ab_tiles[t]
                d = work_pool.tile([P, z], F32, tag="d", bufs=2)
                nc.vector.tensor_sub(
                    d[:], ab[:, off : off + z], ab[:, FREE + off : FREE + off + z]
                )
                w = work_pool.tile([P, z], F32, tag="w", bufs=3)
                nc.scalar.activation(
                    w[:],
                    d[:],
                    mybir.ActivationFunctionType.Square,
                    accum_out=acc[:, i : i + 1],
                )
                w2 = work_pool.tile([P, z], F32, tag="w2", bufs=3)
                nc.scalar.activation(
                    w2[:],
                    w[:],
                    mybir.ActivationFunctionType.Square,
                    accum_out=acc[:, N_PIECES + i : N_PIECES + i + 1],
                )
                w3 = work_pool.tile([P, z], F32, tag="w3", bufs=2)
                nc.vector.scalar_tensor_tensor(
                    out=w3[:],
                    in0=w[:],
                    scalar=1.0,
                    in1=w2[:],
                    op0=mybir.AluOpType.mult,
                    op1=mybir.AluOpType.mult,
                    accum_out=acc[:, 2 * N_PIECES + i : 2 * N_PIECES + i + 1],
                )
            nc.sync.dma_start(out=acc_p[:], in_=acc[:])
    nc.finalize()
    return nc


_NC = None


def _get_nc():
    global _NC
    if _NC is None:
        _NC = _build()
    return _NC


def _shard_inputs(output, target):
    output = np.asarray(output, dtype=np.float32)
    target = np.asarray(target, dtype=np.float32)
    in_maps = []
    for i in range(N_CORES):
        sl = slice(i * ROWS_PER_CORE, (i + 1) * ROWS_PER_CORE)
        o4 = output[sl].reshape(N_TILES, P, FREE)
        t4 = target[sl].reshape(N_TILES, P, FREE)
        comb = np.stack([o4, t4], axis=1).reshape(N_TILES * 2 * P, FREE)
        in_maps.append({"combined": comb})
    return in_maps


def run_device(output, target, trace=False):
    """Returns (per-core partial moment arrays, BassKernelResults)."""
    in_maps = _shard_inputs(output, target)
    res = run_bass_kernel_spmd(_get_nc(), in_maps, list(range(N_CORES)), trace=trace)
    partials = [res.results[i]["partial"] for i in range(N_CORES)]
    return partials, res


def _reduce(partials):
    s1 = s2 = s3 = 0.0
    for p in partials:
        p64 = p.astype(np.float64)
        s1 += p64[:, 0:N_PIECES].sum()
        s2 += p64[:, N_PIECES : 2 * N_PIECES].sum()
        s3 += p64[:, 2 * N_PIECES :].sum()
    c = 4.0 * float(np.float32(1.0 / COLS))  # match reference's f32 sigma
    total = c * s1 - (c * c / 2.0) * s2 + (c * c * c / 6.0) * s3
    n = float(ROWS) * float(COLS)
    return np.array(total / n, dtype=np.float32)


def kernel(output, target):
    partials, _ = run_device(output, target)
    return _reduce(partials)



# revision 2
# speedup vs baseline: 1.4615x; 1.4615x over previous
"""Correntropy loss on 8 Trainium2 NeuronCores - bf16-transport version.

Reference math (all f32):
    t = (target - 0.5) * 2 ; o = (output - 0.5) * 2
    cost = mean(1 - exp(-sigma * (o - t)^2)),  sigma = 1/1000
Since o - t == 2*(output - target), this equals
    mean(1 - exp(-c * w)),  w = (output - target)^2,  c = 4*sigma = 0.004

The problem is HBM-bandwidth bound: 2 x 65536 x 1000 f32 = 524 MB of
input against ~358 GB/s of HBM per core (x8 cores) puts the f32
roofline at ~183 us, which the f32 baseline already hit. The only
remaining lever is moving fewer bytes, so the host rounds both inputs
to bf16 (round-half-up via uint16 view; inputs are [0,1] uniforms, so
no inf/nan edge cases) before staging to device DRAM. That halves HBM
traffic -> ~92 us roofline. The quantization changes the loss by
~2e-6 relative (E[(da-db)^2] ~ 3.5e-7 against E[w]=1/6, and the odd
rounding-error terms average out over 65.5M elements) - far inside
the 2e-2 gate.

Device computes power sums (moments) of w like the f32 baseline, but
only S1 = sum(w) and S2 = sum(w^2); the host evaluates in f64:
    sum(1 - exp(-c*w)) = c*S1 - c^2/2*S2 + O(c^3*S3)
The dropped S3 term is ~6e-7 relative. Engine split per tile:
    DVE: d  = out_half - tgt_half      (tensor_sub, bf16 2x mode)
    ACT: w  = Square(d), accum -> S1   (f32 accumulate)
    DVE: w2 = (w*1)*w,   accum -> S2   (scalar_tensor_tensor, bf16 2x)
At bf16 the per-core budget is DMA 91.5 us vs DVE ~72 us / ACT ~57 us,
so compute pipelines fully under the DMA stream.

Sharding: row-shard into 8 x [8192, 1000]; each core's shards are
host-interleaved per tile as [out-rows ; tgt-rows] so one DMA per tile
fetches both operands. Tiles taper 15x(Q=4 rows/partition) + Q=2 +
2xQ=1 so the serial chain after the final 0.5 MB DMA is short; the
last two Q=1 tiles square on DVE directly (skip S2 there, ~1e-5 of
the correction term) so the tail is sub->stt only.
"""

import numpy as np
import ml_dtypes

import concourse.bacc as bacc
import concourse.mybir as mybir
import concourse.tile as tile
from concourse.bass_utils import run_bass_kernel_spmd

N_CORES = 8
ROWS = 65536
COLS = 1000
ROWS_PER_CORE = ROWS // N_CORES  # 8192
P = 128  # SBUF partitions

# Per-tile rows-per-partition. sum(Q_LIST) * P == ROWS_PER_CORE.
Q_LIST = [4] * 15 + [2, 1, 1]
assert sum(Q_LIST) * P == ROWS_PER_CORE
N_TILES = len(Q_LIST)
# Tiles whose square runs on DVE (stt d*d -> S1) with no S2 moment:
DVE_SQ_TILES = {N_TILES - 2, N_TILES - 1}

F_MAX = max(Q_LIST) * COLS  # 4000
ACC_COLS = 2 * N_TILES  # S1 block | S2 block (S2 cols of DVE_SQ_TILES unused)

BF16 = mybir.dt.bfloat16
F32 = mybir.dt.float32


def _build():
    nc = bacc.Bacc()
    comb_p = nc.declare_dram_parameter(
        "combined", [2 * ROWS_PER_CORE, COLS], BF16, isOutput=False
    )
    acc_p = nc.declare_dram_parameter("partial", [P, ACC_COLS], F32, isOutput=True)

    with tile.TileContext(nc) as tc:
        with (
            tc.tile_pool(name="io", bufs=5) as io_pool,
            tc.tile_pool(name="work", bufs=1) as work_pool,
            tc.tile_pool(name="accp", bufs=1) as acc_pool,
        ):
            acc = acc_pool.tile([P, ACC_COLS], F32)
            r0 = 0
            for t, q in enumerate(Q_LIST):
                f = q * COLS
                nrows = 2 * q * P
                ab = io_pool.tile([P, 2 * F_MAX], BF16, tag="ab")
                src = comb_p[r0 : r0 + nrows, :].rearrange(
                    "(c p q) m -> p c (q m)", c=2, p=P, q=q
                )
                nc.sync.dma_start(
                    out=ab[:, : 2 * f].rearrange("p (c m) -> p c m", c=2), in_=src
                )
                r0 += nrows

                d = work_pool.tile([P, F_MAX], BF16, tag="d", bufs=2)
                nc.vector.tensor_sub(d[:, :f], ab[:, :f], ab[:, f : 2 * f])
                if t in DVE_SQ_TILES:
                    w = work_pool.tile([P, F_MAX], BF16, tag="w", bufs=3)
                    nc.vector.scalar_tensor_tensor(
                        out=w[:, :f],
                        in0=d[:, :f],
                        scalar=1.0,
                        in1=d[:, :f],
                        op0=mybir.AluOpType.mult,
                        op1=mybir.AluOpType.mult,
                        accum_out=acc[:, t : t + 1],
                    )
                else:
                    w = work_pool.tile([P, F_MAX], BF16, tag="w", bufs=3)
                    nc.scalar.activation(
                        w[:, :f],
                        d[:, :f],
                        mybir.ActivationFunctionType.Square,
                        accum_out=acc[:, t : t + 1],
                    )
                    w2 = work_pool.tile([P, F_MAX], BF16, tag="w2", bufs=2)
                    nc.vector.scalar_tensor_tensor(
                        out=w2[:, :f],
                        in0=w[:, :f],
                        scalar=1.0,
                        in1=w[:, :f],
                        op0=mybir.AluOpType.mult,
                        op1=mybir.AluOpType.mult,
                        accum_out=acc[:, N_TILES + t : N_TILES + t + 1],
                    )
            nc.sync.dma_start(out=acc_p[:], in_=acc[:])
    nc.finalize()
    return nc


_NC = None


def _get_nc():
    global _NC
    if _NC is None:
        _NC = _build()
    return _NC


def _to_bf16(a):
    """Round-half-up f32 -> bf16 via integer view (one pass, unbiased for
    continuous data; inputs are finite and positive so no carry into inf)."""
    v = np.asarray(a, dtype=np.float32).view(np.uint32)
    return ((v + np.uint32(0x8000)) >> np.uint16(16)).astype(np.uint16).view(
        ml_dtypes.bfloat16
    )


def _shard_inputs(output, target):
    ob = _to_bf16(output)
    tb = _to_bf16(target)
    in_maps = []
    for i in range(N_CORES):
        base = i * ROWS_PER_CORE
        blocks = []
        r0 = 0
        for q in Q_LIST:
            nrows = q * P
            blocks.append(ob[base + r0 : base + r0 + nrows])
            blocks.append(tb[base + r0 : base + r0 + nrows])
            r0 += nrows
        in_maps.append({"combined": np.concatenate(blocks, axis=0)})
    return in_maps


def run_device(output, target, trace=False):
    """Returns (per-core partial moment arrays, BassKernelResults)."""
    in_maps = _shard_inputs(output, target)
    res = run_bass_kernel_spmd(_get_nc(), in_maps, list(range(N_CORES)), trace=trace)
    partials = [res.results[i]["partial"] for i in range(N_CORES)]
    return partials, res


def _reduce(partials):
    s1 = s2 = 0.0
    s2_cols = [N_TILES + t for t in range(N_TILES) if t not in DVE_SQ_TILES]
    for p in partials:
        p64 = p.astype(np.float64)
        s1 += p64[:, 0:N_TILES].sum()
        s2 += p64[:, s2_cols].sum()
    c = 4.0 * float(np.float32(1.0 / COLS))  # match reference's f32 sigma
    total = c * s1 - (c * c / 2.0) * s2
    n = float(ROWS) * float(COLS)
    return np.array(total / n, dtype=np.float32)


def kernel(output, target):
    partials, _ = run_device(output, target)
    return _reduce(partials)


# revision 10
# speedup vs baseline: 2.0560x; 1.4068x over previous
"""Correntropy loss on 8 Trainium2 NeuronCores - bf16-transport version.

Reference math (all f32):
    t = (target - 0.5) * 2 ; o = (output - 0.5) * 2
    cost = mean(1 - exp(-sigma * (o - t)^2)),  sigma = 1/1000
Since o - t == 2*(output - target), this equals
    mean(1 - exp(-c * w)),  w = (output - target)^2,  c = 4*sigma = 0.004

The problem is HBM-bandwidth bound: 2 x 65536 x 1000 f32 = 524 MB of
input against ~358 GB/s of HBM per core (x8 cores) puts the f32
roofline at ~183 us, which the f32 baseline already hit. The only
remaining lever is moving fewer bytes, so the host rounds both inputs
to bf16 (round-half-up via uint16 view; inputs are [0,1] uniforms, so
no inf/nan edge cases) before staging to device DRAM. That halves HBM
traffic -> ~92 us roofline. The quantization changes the loss by
~2e-6 relative (E[(da-db)^2] ~ 3.5e-7 against E[w]=1/6, and the odd
rounding-error terms average out over 65.5M elements) - far inside
the 2e-2 gate.

Device computes power sums (moments) of w like the f32 baseline, but
only S1 = sum(w) and S2 = sum(w^2); the host evaluates in f64:
    sum(1 - exp(-c*w)) = c*S1 - c^2/2*S2 + O(c^3*S3)
The dropped S3 term is ~6e-7 relative. Engine split per tile:
    DVE: d  = out_half - tgt_half      (tensor_sub, bf16 2x mode)
    ACT: w  = Square(d), accum -> S1   (f32 accumulate)
    DVE: w2 = (w*1)*w,   accum -> S2   (scalar_tensor_tensor, bf16 2x)
At bf16 the per-core budget is DMA 91.5 us vs DVE ~72 us / ACT ~57 us,
so compute pipelines fully under the DMA stream.

Sharding: row-shard into 8 x [8192, 1000]; each core's shards are
host-interleaved per tile as [out-rows ; tgt-rows] so one DMA per tile
fetches both operands. Tiles taper 15x(Q=4 rows/partition) + Q=2 +
2xQ=1 so the serial chain after the final 0.5 MB DMA is short; the
last two Q=1 tiles square on DVE directly (skip S2 there, ~1e-5 of
the correction term) so the tail is sub->stt only.
"""

import numpy as np
import ml_dtypes

import concourse.bacc as bacc
import concourse.mybir as mybir
import concourse.tile as tile
from concourse.bass_utils import run_bass_kernel_spmd

N_CORES = 8
ROWS = 65536
COLS = 1000
ROWS_PER_CORE = ROWS // N_CORES  # 8192
P = 128  # SBUF partitions

# Per-tile rows-per-partition. sum(Q_LIST) * P == ROWS_PER_CORE.
Q_LIST = [4] * 15 + [2, 1, 1]
assert sum(Q_LIST) * P == ROWS_PER_CORE
N_TILES = len(Q_LIST)
# Engine assignment per tile, balancing busy time under the ~78 us DMA
# stream (trace-measured: DVE sub 2.24us/Q4-tile at bf16 2x; ACT square
# 3.63+0.28us; stt on DVE runs 1x = 4.32us; gpsimd stt assumed ~3.4us):
#   S1 square: ACT for most tiles; DVE for 2 mid tiles + the 2 tail Q1
#   tiles (short tail chain); gpsimd for 2 mid tiles.
#   S2 square: gpsimd for all moment tiles except the DVE-S1 ones (the
#   dropped S2 slivers shift the series term by ~1e-4 of itself, i.e.
#   ~1e-7 of the loss).
# S1 squares: ACT for most tiles; DVE stt (1x) for 3 mid tiles to
# balance, plus the 2 tail Q1 tiles (short tail chain).
S1_DVE_TILES = {4, 9, 14, N_TILES - 2, N_TILES - 1}
# S2 is a ~8e-4 relative correction; sampling it on 2 of the 16 moment
# tiles (8 of 64 Q-units -> host scales by 64/8) adds ~1e-6 noise.
S2_TILES = {2, 7}
S2_SAMPLE_SCALE = float(sum(Q_LIST)) / float(sum(Q_LIST[t] for t in S2_TILES))

F_MAX = max(Q_LIST) * COLS  # 4000
ACC_COLS = 2 * N_TILES  # S1 block | S2 block (S2 cols of DVE_SQ_TILES unused)

BF16 = mybir.dt.bfloat16
F32 = mybir.dt.float32


def _build():
    nc = bacc.Bacc()
    comb_p = nc.declare_dram_parameter(
        "combined", [2 * ROWS_PER_CORE, COLS], BF16, isOutput=False
    )
    acc_p = nc.declare_dram_parameter("partial", [P, ACC_COLS], F32, isOutput=True)

    with tile.TileContext(nc) as tc:
        with (
            tc.tile_pool(name="io", bufs=6) as io_pool,
            tc.tile_pool(name="work", bufs=1) as work_pool,
            tc.tile_pool(name="accp", bufs=1) as acc_pool,
        ):
            acc = acc_pool.tile([P, ACC_COLS], F32)

            def stt_square(eng, out_ap, in_ap, acc_ap):
                eng.scalar_tensor_tensor(
                    out=out_ap,
                    in0=in_ap,
                    scalar=1.0,
                    in1=in_ap,
                    op0=mybir.AluOpType.mult,
                    op1=mybir.AluOpType.mult,
                    accum_out=acc_ap,
                )

            r0 = 0
            for t, q in enumerate(Q_LIST):
                f = q * COLS
                nrows = 2 * q * P
                ab = io_pool.tile([P, 2 * F_MAX], BF16, tag="ab")
                src = comb_p[r0 : r0 + nrows, :].rearrange(
                    "(c p q) m -> p c (q m)", c=2, p=P, q=q
                )
                nc.sync.dma_start(
                    out=ab[:, : 2 * f].rearrange("p (c m) -> p c m", c=2), in_=src
                )
                r0 += nrows

                d = work_pool.tile([P, F_MAX], BF16, tag="d", bufs=2)
                nc.vector.tensor_sub(d[:, :f], ab[:, :f], ab[:, f : 2 * f])
                w = work_pool.tile([P, F_MAX], BF16, tag="w", bufs=3)
                s1col = acc[:, t : t + 1]
                if t in S1_DVE_TILES:
                    stt_square(nc.vector, w[:, :f], d[:, :f], s1col)
                else:
                    nc.scalar.activation(
                        w[:, :f],
                        d[:, :f],
                        mybir.ActivationFunctionType.Square,
                        accum_out=s1col,
                    )
                if t in S2_TILES:
                    w2 = work_pool.tile([P, F_MAX], BF16, tag="w2", bufs=2)
                    nc.scalar.activation(
                        w2[:, :f],
                        w[:, :f],
                        mybir.ActivationFunctionType.Square,
                        accum_out=acc[:, N_TILES + t : N_TILES + t + 1],
                    )
            nc.sync.dma_start(out=acc_p[:], in_=acc[:])
    nc.finalize()
    return nc


_NC = None


def _get_nc():
    global _NC
    if _NC is None:
        _NC = _build()
    return _NC


def _to_bf16(a):
    """Round-half-up f32 -> bf16 via integer view (one pass, unbiased for
    continuous data; inputs are finite and positive so no carry into inf)."""
    v = np.asarray(a, dtype=np.float32).view(np.uint32)
    return ((v + np.uint32(0x8000)) >> np.uint16(16)).astype(np.uint16).view(
        ml_dtypes.bfloat16
    )


def _shard_inputs(output, target):
    ob = _to_bf16(output)
    tb = _to_bf16(target)
    in_maps = []
    for i in range(N_CORES):
        base = i * ROWS_PER_CORE
        blocks = []
        r0 = 0
        for q in Q_LIST:
            nrows = q * P
            blocks.append(ob[base + r0 : base + r0 + nrows])
            blocks.append(tb[base + r0 : base + r0 + nrows])
            r0 += nrows
        in_maps.append({"combined": np.concatenate(blocks, axis=0)})
    return in_maps


def run_device(output, target, trace=False):
    """Returns (per-core partial moment arrays, BassKernelResults)."""
    in_maps = _shard_inputs(output, target)
    res = run_bass_kernel_spmd(_get_nc(), in_maps, list(range(N_CORES)), trace=trace)
    partials = [res.results[i]["partial"] for i in range(N_CORES)]
    return partials, res


def _reduce(partials):
    s1 = s2 = 0.0
    s2_cols = [N_TILES + t for t in sorted(S2_TILES)]
    for p in partials:
        p64 = p.astype(np.float64)
        s1 += p64[:, 0:N_TILES].sum()
        s2 += p64[:, s2_cols].sum()
    s2 *= S2_SAMPLE_SCALE
    c = 4.0 * float(np.float32(1.0 / COLS))  # match reference's f32 sigma
    total = c * s1 - (c * c / 2.0) * s2
    n = float(ROWS) * float(COLS)
    return np.array(total / n, dtype=np.float32)


def kernel(output, target):
    partials, _ = run_device(output, target)
    return _reduce(partials)
